# revision 15
# baseline (speedup 1.0000x reference)
"""Multi-head attention kernel for Trainium2, 8 NeuronCores.

Problem (NHEAD=8, T=S=1024, B=8, A=512, hd=64):
  q = queries.reshape(T, B*NH, hd); k = keys.reshape(S, B*NH, hd)
  w = softmax(mask(q @ k^T / sqrt(hd)))      per n = b*NH + h, mask = attn_mask[n % NH]
  out = (w @ k).reshape(T, B, A)             (keys double as values)

Sharding: head-parallel. Core c owns head h=c for all 8 batches; every
problem on core c uses the single mask slice attn_mask[c] (n % 8 == h).

Per-core dataflow (all matmul operands bf16, f32 PSUM accumulation):
  - qT/kT [h, t] layouts prepared on host; two batches per 128-partition
    tile (batch 2p on partitions 0-63, 2p+1 on 64-127).
  - mm1 scores are packed per (s_tile, th-chunk): sc[128, 2, 512] where
    the two 512-wide column blocks (= two PSUM banks) hold batch b0/b1
    scores for the same 512 t-columns.  The two K=64 matmuls use PE row
    tiling (tile_position (0,0)/(64,0)) AND disjoint PSUM banks, so they
    run concurrently on the array.
  - ACT: pe = exp(sc * 1/8) PSUM->SBUF bf16 over both batches at once
    (no max subtraction needed: |scores/8| <= ~6).  ACT is the
    bottleneck engine (64 x ~1.05us EXPs = 67us); everything else is
    scheduled to hide behind it.
  - DVE: pt = pe * maskT, one 512-wide bf16 2x-mode multiply per batch
    (mask is per-head, shared by both batches).
  - mm2: out[t_tile, 65] += pt.T @ [k | ones]; column 64 accumulates the
    softmax denominator. 16 (tt, b01) 65-wide blocks per pair live in 3
    persistent PSUM banks.  mm2 emission is skewed one (pair, st) stage
    behind mm1/exp/mask so the next stage's matmuls are never queued
    behind mm2's pt-wait on the strict-FIFO PE queue.
  - normalize: per PSUM bank-group, one reciprocal + one broadcast
    multiply into the output tile; output DMA'd per batch-pair (dense
    [B, T, HD] DRAM layout) so only the last pair's store is on the tail.
  - DMA issue latency (~0.7us per dma_start on the issuing queue) gates
    the head: kt0 is issued from the Activation queue in parallel with
    qt0 on the SP queue so mm1 can start right after the preamble; bulk
    qt/kt prefetch for pairs 1-3 goes through GpSimd SWDGE queues to
    keep the SP queue free for the mask/kn stream pair 0 consumes.
"""

import numpy as np
import ml_dtypes

import concourse.bass as bass
import concourse.mybir as mybir
import concourse.tile as tile
from concourse.bass_utils import run_bass_kernel_spmd

BF16 = ml_dtypes.bfloat16

T = 1024
S = 1024
B = 8
NH = 8
HD = 64
N_CORES = 8
SCALE = 1.0 / 8.0  # 1/sqrt(hd)


# Empirical per-instruction sem-wait limit for this walrus build: even a
# Matmult with 2 waits fails codegen ("Too many sync wait commands"), so
# every instruction keeps at most one inline wait.
_WAIT_LIMITS = {}


def _split_excess_waits(nc, default_max=1):
    """Hoist excess sem waits off instructions onto standalone
    EventSemaphore waits placed just before them on the same engine queue —
    semantically identical, since each engine executes its queue in order."""
    n = 0
    for f in nc.m.functions:
        for bb in f.blocks:
            insts = bb.instructions
            out = []
            changed = False
            for ins in insts:
                si = ins.sync_info
                waits = list(si.on_wait) if si is not None and si.on_wait else []
                max_waits = _WAIT_LIMITS.get(type(ins).__name__, default_max)
                if (
                    len(waits) > max_waits
                    and type(ins).__name__ != "InstEventSemaphore"
                ):
                    changed = True
                    for w in waits[:-max_waits]:
                        n += 1
                        we = mybir.InstEventSemaphore(
                            name=f"WSPLIT-{n}", ins=[], outs=[]
                        )
                        we.engine = ins.engine
                        we.sync_info = mybir.SyncInfo(on_wait=[w], on_update=[])
                        nc.register_instruction(we)
                        out.append(we)
                    ins.sync_info = mybir.SyncInfo(
                        on_wait=waits[-max_waits:],
                        on_update=list(si.on_update) if si.on_update else [],
                    )
                out.append(ins)
            if changed:
                bb.instructions = out


def build_nc():
    fp32 = mybir.dt.float32
    bf16 = mybir.dt.bfloat16

    nc = bass.Bass(target_bir_lowering=False)
    # Per-core inputs (host pre-sliced/cast/transposed; SPMD: same program,
    # per-core data). qt/kt rows are (b, h) pairs: rows 128p..128p+127 hold
    # batches 2p (partitions 0-63) and 2p+1 (partitions 64-127).
    qt_in = nc.dram_tensor("qt", [B * HD, T], bf16, kind="ExternalInput")
    kt_in = nc.dram_tensor("kt", [B * HD, S], bf16, kind="ExternalInput")
    knat = nc.dram_tensor("knat", [S, B * HD], bf16, kind="ExternalInput")
    maskt = nc.dram_tensor("maskt", [S, T], bf16, kind="ExternalInput")
    # dense per-batch output layout so the per-pair store is a contiguous
    # 512 KiB DRAM range (the host transposes back).
    out = nc.dram_tensor("out", [B, T, HD], fp32, kind="ExternalOutput")

    knat3 = knat.rearrange("(st p) (b h) -> st p b h", p=128, b=B)
    out4 = out.rearrange("b (tt p) h -> p tt b h", p=128)

    with tile.TileContext(nc) as tc:
        with (
            tc.tile_pool(name="consts", bufs=1) as consts,
            tc.tile_pool(name="ptp", bufs=6) as ptp,
            tc.tile_pool(name="pte", bufs=3) as pte,
            tc.tile_pool(name="rcp", bufs=4) as rcp,
            tc.tile_pool(name="scp", bufs=2, space="PSUM") as scp,
            tc.tile_pool(name="opp", bufs=1, space="PSUM") as opp,
        ):
            kt = [consts.tile([128, S], bf16, tag=f"kt{p}", name=f"kt{p}") for p in range(4)]
            # kt0 DMA issued from the Activation queue so it runs in
            # parallel with qt0's issue on the SP queue (DIRECT2D issue is
            # ~0.7us of sequencer time each; serialized they gate mm1).
            nc.scalar.dma_start(out=kt[0][:], in_=kt_in[0:128, :])

            # warm the ACT exp table immediately after (also on the
            # Activation queue) so the ~2.7us table load overlaps the DMAs.
            wsrc = consts.tile([128, 1], mybir.dt.float32, tag="wsrc", name="wsrc")
            wdst = consts.tile([128, 1], bf16, tag="wdst", name="wdst")
            nc.vector.memset(wsrc[:], 0.0)
            nc.scalar.activation(
                wdst[:], wsrc[:], mybir.ActivationFunctionType.Exp
            )

            # --- resident tiles, DMA'd in consumption order ----------------
            qt = [consts.tile([128, T], bf16, tag=f"qt{p}", name=f"qt{p}") for p in range(4)]
            mt = [consts.tile([128, T], bf16, tag=f"mt{s}", name=f"mt{s}") for s in range(8)]
            kn = [
                consts.tile([128, B, HD + 1], bf16, tag=f"kn{s}", name=f"kn{s}")
                for s in range(8)
            ]
            outt = consts.tile([128, 8, B, HD], fp32, tag="outt", name="outt")

            # SP HWDGE queue order IS the DMA service order (each queue's
            # descriptors drain FIFO), so pair 0's mask/kn stream goes
            # first and pair p's qt/kt prefetch is interleaved late enough
            # not to starve it but early enough to land before pair p.
            nc.sync.dma_start(out=qt[0][:, 0:512], in_=qt_in[0:128, 0:512])
            nc.sync.dma_start(out=qt[0][:, 512:1024], in_=qt_in[0:128, 512:1024])
            prefetch_after = {2: 1, 5: 2, 7: 3}
            for st in range(8):
                nc.sync.dma_start(
                    out=mt[st][:], in_=maskt[st * 128 : (st + 1) * 128, :]
                )
                nc.vector.memset(kn[st][:, :, HD], 1.0)
                nc.sync.dma_start(out=kn[st][:, :, 0:HD], in_=knat3[st])
                p = prefetch_after.get(st)
                if p is not None:
                    nc.sync.dma_start(out=kt[p][:], in_=kt_in[p * 128 : (p + 1) * 128, :])
                    nc.sync.dma_start(out=qt[p][:], in_=qt_in[p * 128 : (p + 1) * 128, :])

            # --- main loop, software-pipelined by one (pair, st) stage ----
            # stage g: mm1/exp/mask for tile g; mm2 for tile g-1.  This
            # keeps the next tile's mm1 ahead of mm2's pt-wait in the
            # strict-FIFO PE queue (mm2 waits on the DVE mask output).
            ops = [None] * 4  # per-pair PSUM accumulators, allocated lazily

            def emit_front(g):
                pair, st = divmod(g, 8)
                pts = []
                for th in range(2):
                    sc = scp.tile(
                        [128, 2, 512], fp32, tag="sc",
                        name=f"sc_{pair}_{st}_{th}",
                    )
                    for b01 in range(2):
                        lhsT = kt[pair][
                            b01 * 64 : (b01 + 1) * 64, st * 128 : (st + 1) * 128
                        ]
                        rhs = qt[pair][
                            b01 * 64 : (b01 + 1) * 64, th * 512 : (th + 1) * 512
                        ]
                        nc.tensor.matmul(
                            sc[:, b01, :],
                            lhsT,
                            rhs,
                            start=True,
                            stop=True,
                            tile_position=(b01 * 64, 0),
                        )
                    pe = pte.tile(
                        [128, 2, 512], bf16, tag="pe",
                        name=f"pe_{pair}_{st}_{th}",
                    )
                    nc.scalar.activation(
                        pe[:], sc[:], mybir.ActivationFunctionType.Exp,
                        scale=SCALE,
                    )
                    pt = ptp.tile(
                        [128, 2, 512], bf16, tag="pt",
                        name=f"pt_{pair}_{st}_{th}",
                    )
                    for b01 in range(2):
                        nc.vector.tensor_tensor(
                            out=pt[:, b01, :], in0=pe[:, b01, :],
                            in1=mt[st][:, th * 512 : (th + 1) * 512],
                            op=mybir.AluOpType.mult,
                        )
                    pts.append(pt)
                return pts

            def emit_mm2(g, pts):
                # mm2 contributions of s_tile `st` for every t_tile.
                # start=True clears the WHOLE PSUM bank, so only the
                # chronologically first matmul into each op tile (per
                # pair) may carry it; later blocks in the same bank
                # initialize via per-element has_written bits.
                pair, st = divmod(g, 8)
                if st == 0:
                    ops[pair] = [
                        opp.tile([128, 512], fp32, tag=f"op{j}", name=f"op{j}_{pair}")
                        for j in range(3)
                    ]
                for th in range(2):
                    for k in range(4):
                        tt = th * 4 + k
                        j, loc = tt // 3, tt % 3
                        for b01 in range(2):
                            b = pair * 2 + b01
                            nc.tensor.matmul(
                                ops[pair][j][
                                    :,
                                    loc * 130 + b01 * 65 : loc * 130 + (b01 + 1) * 65,
                                ],
                                pts[th][:, b01, k * 128 : (k + 1) * 128],
                                kn[st][:, b, :],
                                start=(st == 0 and loc == 0 and b01 == 0),
                                stop=(st == 7),
                                skip_group_check=True,
                            )

            def emit_normalize(pair):
                # one reciprocal + one broadcast multiply per PSUM
                # bank-group (vs per-tt), writing the output tile, then
                # stream this pair's output so only pair 3's store is on
                # the kernel tail.  Pair 3's store is further split per
                # bank-group across both HWDGE queues so it streams out
                # while the remaining groups normalize.
                for j in range(3):
                    nloc = 3 if j < 2 else 2
                    opv = ops[pair][j][:, 0 : nloc * 130].rearrange(
                        "p (l b x) -> p l b x", l=nloc, b=2
                    )
                    rc = rcp.tile(
                        [128, nloc, 2, 1], fp32, tag=f"rc{j}",
                        name=f"rc_{pair}_{j}",
                    )
                    nc.vector.reciprocal(rc[:, :, :, 0], opv[:, :, :, HD])
                    nc.vector.tensor_tensor(
                        out=outt[:, j * 3 : j * 3 + nloc, pair * 2 : (pair + 1) * 2, :],
                        in0=opv[:, :, :, 0:HD],
                        in1=rc[:].to_broadcast([128, nloc, 2, HD]),
                        op=mybir.AluOpType.mult,
                    )
                    if pair == 3:
                        for b01 in range(2):
                            b = pair * 2 + b01
                            eng = nc.sync if b01 == 0 else nc.scalar
                            eng.dma_start(
                                out=out4[:, j * 3 : j * 3 + nloc, b, :],
                                in_=outt[:, j * 3 : j * 3 + nloc, b, :],
                            )
                if pair < 3:
                    for b01 in range(2):
                        b = pair * 2 + b01
                        nc.sync.dma_start(
                            out=out4[:, :, b, :],
                            in_=outt[:, :, b, :],
                        )

            prev_pts = None
            for g in range(33):
                pts = emit_front(g) if g < 32 else None
                if g >= 1:
                    emit_mm2(g - 1, prev_pts)
                    if g % 8 == 0:
                        emit_normalize(g // 8 - 1)
                prev_pts = pts

    _split_excess_waits(nc)
    return nc


_NC_CACHE = None


def _get_nc():
    global _NC_CACHE
    if _NC_CACHE is None:
        _NC_CACHE = build_nc()
    return _NC_CACHE


def kernel(queries: np.ndarray, keys: np.ndarray, attn_mask: np.ndarray) -> np.ndarray:
    assert queries.shape == (T, B, NH * HD)
    assert keys.shape == (S, B, NH * HD)
    assert attn_mask.shape == (B, T, S)

    q_bf = np.asarray(queries, np.float32).astype(BF16)  # [T, B, A]
    k_bf = np.asarray(keys, np.float32).astype(BF16)
    m_bf = np.asarray(attn_mask).astype(BF16)  # bool -> 0.0/1.0

    in_maps = []
    for c in range(N_CORES):
        qs = q_bf[:, :, c * HD : (c + 1) * HD].reshape(T, B * HD)  # [T,(b,h)]
        ks = k_bf[:, :, c * HD : (c + 1) * HD].reshape(S, B * HD)
        in_maps.append(
            {
                "qt": np.ascontiguousarray(qs.T),
                "kt": np.ascontiguousarray(ks.T),
                "knat": np.ascontiguousarray(ks),
                "maskt": np.ascontiguousarray(m_bf[c].T),
            }
        )

    nc = _get_nc()
    res = run_bass_kernel_spmd(nc, in_maps, core_ids=list(range(N_CORES)))
    kernel.last_results = res

    outp = np.empty((T, B, NH * HD), np.float32)
    for c in range(N_CORES):
        # device output is [B, T, HD]; put back as [T, B, HD-slice]
        outp[:, :, c * HD : (c + 1) * HD] = res.results[c]["out"].transpose(1, 0, 2)
    return outp


# revision 18
# speedup vs baseline: 1.1760x; 1.1760x over previous
"""Multi-head attention kernel for Trainium2, 8 NeuronCores.

Problem (NHEAD=8, T=S=1024, B=8, A=512, hd=64):
  q = queries.reshape(T, B*NH, hd); k = keys.reshape(S, B*NH, hd)
  w = softmax(mask(q @ k^T / sqrt(hd)))      per n = b*NH + h, mask = attn_mask[n % NH]
  out = (w @ k).reshape(T, B, A)             (keys double as values)

Sharding: head-parallel. Core c owns head h=c for all 8 batches; every
problem on core c uses the single mask slice attn_mask[c] (n % 8 == h).

Per-core dataflow (all matmul operands bf16, f32 PSUM accumulation):
  - qT/kT [h, t] layouts prepared on host; two batches per 128-partition
    tile (batch 2p on partitions 0-63, 2p+1 on 64-127).
  - mm1 scores are packed per (s_tile, th-chunk): sc[128, 2, 512] where
    the two 512-wide column blocks (= two PSUM banks) hold batch b0/b1
    scores for the same 512 t-columns.  The two K=64 matmuls use PE row
    tiling (tile_position (0,0)/(64,0)) AND disjoint PSUM banks, so they
    run concurrently on the array.
  - ACT: pe = exp(sc * 1/8) PSUM->SBUF bf16 over both batches at once
    (no max subtraction needed: |scores/8| <= ~6).  ACT is the
    bottleneck engine (64 x ~1.05us EXPs = 67us); everything else is
    scheduled to hide behind it.
  - DVE: pt = pe * maskT, one 512-wide bf16 2x-mode multiply per batch
    (mask is per-head, shared by both batches).
  - mm2: out[t_tile, 65] += pt.T @ [k | ones]; column 64 accumulates the
    softmax denominator. 16 (tt, b01) 65-wide blocks per pair live in 3
    persistent PSUM banks.  mm2 emission is skewed one (pair, st) stage
    behind mm1/exp/mask so the next stage's matmuls are never queued
    behind mm2's pt-wait on the strict-FIFO PE queue.
  - normalize: per PSUM bank-group, one reciprocal + one broadcast
    multiply into the output tile; output DMA'd per batch-pair (dense
    [B, T, HD] DRAM layout) so only the last pair's store is on the tail.
  - DMA issue latency (~0.7us per dma_start on the issuing queue) gates
    the head: kt0 is issued from the Activation queue in parallel with
    qt0 on the SP queue so mm1 can start right after the preamble; bulk
    qt/kt prefetch for pairs 1-3 goes through GpSimd SWDGE queues to
    keep the SP queue free for the mask/kn stream pair 0 consumes.
"""

import numpy as np
import ml_dtypes

import concourse.bass as bass
import concourse.mybir as mybir
import concourse.tile as tile
from concourse.bass_utils import run_bass_kernel_spmd

BF16 = ml_dtypes.bfloat16

T = 1024
S = 1024
B = 8
NH = 8
HD = 64
N_CORES = 8
SCALE = 1.0 / 8.0  # 1/sqrt(hd)


# Empirical per-instruction sem-wait limit for this walrus build: even a
# Matmult with 2 waits fails codegen ("Too many sync wait commands"), so
# every instruction keeps at most one inline wait.
_WAIT_LIMITS = {}


def _split_excess_waits(nc, default_max=1):
    """Hoist excess sem waits off instructions onto standalone
    EventSemaphore waits placed just before them on the same engine queue —
    semantically identical, since each engine executes its queue in order."""
    n = 0
    for f in nc.m.functions:
        for bb in f.blocks:
            insts = bb.instructions
            out = []
            changed = False
            for ins in insts:
                si = ins.sync_info
                waits = list(si.on_wait) if si is not None and si.on_wait else []
                max_waits = _WAIT_LIMITS.get(type(ins).__name__, default_max)
                if (
                    len(waits) > max_waits
                    and type(ins).__name__ != "InstEventSemaphore"
                ):
                    changed = True
                    for w in waits[:-max_waits]:
                        n += 1
                        we = mybir.InstEventSemaphore(
                            name=f"WSPLIT-{n}", ins=[], outs=[]
                        )
                        we.engine = ins.engine
                        we.sync_info = mybir.SyncInfo(on_wait=[w], on_update=[])
                        nc.register_instruction(we)
                        out.append(we)
                    ins.sync_info = mybir.SyncInfo(
                        on_wait=waits[-max_waits:],
                        on_update=list(si.on_update) if si.on_update else [],
                    )
                out.append(ins)
            if changed:
                bb.instructions = out


def build_nc():
    fp32 = mybir.dt.float32
    bf16 = mybir.dt.bfloat16

    nc = bass.Bass(target_bir_lowering=False)
    # Per-core inputs (host pre-sliced/cast/transposed; SPMD: same program,
    # per-core data). qt/kt rows are (b, h) pairs: rows 128p..128p+127 hold
    # batches 2p (partitions 0-63) and 2p+1 (partitions 64-127).
    qt_in = nc.dram_tensor("qt", [B * HD, T], bf16, kind="ExternalInput")
    kt_in = nc.dram_tensor("kt", [B * HD, S], bf16, kind="ExternalInput")
    knat = nc.dram_tensor("knat", [S, B * HD], bf16, kind="ExternalInput")
    maskt = nc.dram_tensor("maskt", [S, T], bf16, kind="ExternalInput")
    # dense per-batch output layout so the per-pair store is a contiguous
    # 512 KiB DRAM range (the host transposes back).
    out = nc.dram_tensor("out", [B, T, HD], fp32, kind="ExternalOutput")

    knat3 = knat.rearrange("(st p) (b h) -> st p b h", p=128, b=B)
    out4 = out.rearrange("b (tt p) h -> p tt b h", p=128)

    with tile.TileContext(nc) as tc:
        with (
            tc.tile_pool(name="consts", bufs=1) as consts,
            tc.tile_pool(name="ptp", bufs=6) as ptp,
            # 4 pe slots: with 3, EXP(st2,th1) of each pair stalls on a
            # mask op queued behind the previous pair's normalize on DVE.
            tc.tile_pool(name="pte", bufs=4) as pte,
            tc.tile_pool(name="rcp", bufs=4) as rcp,
            tc.tile_pool(name="scp", bufs=2, space="PSUM") as scp,
            tc.tile_pool(name="opp", bufs=1, space="PSUM") as opp,
        ):
            kt = [consts.tile([128, S], bf16, tag=f"kt{p}", name=f"kt{p}") for p in range(4)]

            # warm the ACT exp table first on the Activation queue (the
            # ~2.7us table load overlaps the input DMAs), then issue the
            # bulk of kt0 from the same queue so it transfers in parallel
            # with the SP-queue stream.
            wsrc = consts.tile([128, 1], mybir.dt.float32, tag="wsrc", name="wsrc")
            wdst = consts.tile([128, 1], bf16, tag="wdst", name="wdst")
            nc.vector.memset(wsrc[:], 0.0)
            nc.scalar.activation(
                wdst[:], wsrc[:], mybir.ActivationFunctionType.Exp
            )
            nc.scalar.dma_start(out=kt[0][:, 128:512], in_=kt_in[0:128, 128:512])
            nc.scalar.dma_start(out=kt[0][:, 512:1024], in_=kt_in[0:128, 512:1024])

            # --- resident tiles, DMA'd in consumption order ----------------
            qt = [consts.tile([128, T], bf16, tag=f"qt{p}", name=f"qt{p}") for p in range(4)]
            mt = [consts.tile([128, T], bf16, tag=f"mt{s}", name=f"mt{s}") for s in range(8)]
            kn = [
                consts.tile([128, B, HD + 1], bf16, tag=f"kn{s}", name=f"kn{s}")
                for s in range(8)
            ]
            outt = consts.tile([128, 8, B, HD], fp32, tag="outt", name="outt")

            # SP HWDGE queue order IS the DMA service order (each queue's
            # descriptors drain FIFO), so pair 0's mask/kn stream goes
            # first and pair p's qt/kt prefetch is interleaved late enough
            # not to starve it but early enough to land before pair p.
            # mm1(st0) needs only kt0's first 128 columns; that 32 KiB
            # slice leads the SP queue so the first matmul isn't gated on
            # the full 256 KiB kt0 transfer.
            nc.sync.dma_start(out=kt[0][:, 0:128], in_=kt_in[0:128, 0:128])
            nc.sync.dma_start(out=qt[0][:, 0:512], in_=qt_in[0:128, 0:512])
            nc.sync.dma_start(out=qt[0][:, 512:1024], in_=qt_in[0:128, 512:1024])
            prefetch_after = {2: 1, 5: 2, 7: 3}
            for st in range(8):
                nc.sync.dma_start(
                    out=mt[st][:], in_=maskt[st * 128 : (st + 1) * 128, :]
                )
                nc.vector.memset(kn[st][:, :, HD], 1.0)
                nc.sync.dma_start(out=kn[st][:, :, 0:HD], in_=knat3[st])
                p = prefetch_after.get(st)
                if p is not None:
                    nc.sync.dma_start(out=kt[p][:], in_=kt_in[p * 128 : (p + 1) * 128, :])
                    nc.sync.dma_start(out=qt[p][:], in_=qt_in[p * 128 : (p + 1) * 128, :])

            # --- main loop, software-pipelined by one (pair, st) stage ----
            # stage g: mm1/exp/mask for tile g; mm2 for tile g-1.  This
            # keeps the next tile's mm1 ahead of mm2's pt-wait in the
            # strict-FIFO PE queue (mm2 waits on the DVE mask output).
            ops = [None] * 4  # per-pair PSUM accumulators, allocated lazily

            def emit_front(g):
                pair, st = divmod(g, 8)
                pts = []
                for th in range(2):
                    sc = scp.tile(
                        [128, 2, 512], fp32, tag="sc",
                        name=f"sc_{pair}_{st}_{th}",
                    )
                    for b01 in range(2):
                        lhsT = kt[pair][
                            b01 * 64 : (b01 + 1) * 64, st * 128 : (st + 1) * 128
                        ]
                        rhs = qt[pair][
                            b01 * 64 : (b01 + 1) * 64, th * 512 : (th + 1) * 512
                        ]
                        nc.tensor.matmul(
                            sc[:, b01, :],
                            lhsT,
                            rhs,
                            start=True,
                            stop=True,
                            tile_position=(b01 * 64, 0),
                        )
                    pe = pte.tile(
                        [128, 2, 512], bf16, tag="pe",
                        name=f"pe_{pair}_{st}_{th}",
                    )
                    nc.scalar.activation(
                        pe[:], sc[:], mybir.ActivationFunctionType.Exp,
                        scale=SCALE,
                    )
                    pt = ptp.tile(
                        [128, 2, 512], bf16, tag="pt",
                        name=f"pt_{pair}_{st}_{th}",
                    )
                    for b01 in range(2):
                        nc.vector.tensor_tensor(
                            out=pt[:, b01, :], in0=pe[:, b01, :],
                            in1=mt[st][:, th * 512 : (th + 1) * 512],
                            op=mybir.AluOpType.mult,
                        )
                    pts.append(pt)
                return pts

            def emit_mm2(g, pts):
                # mm2 contributions of s_tile `st` for every t_tile.
                # start=True clears the WHOLE PSUM bank, so only the
                # chronologically first matmul into each op tile (per
                # pair) may carry it; later blocks in the same bank
                # initialize via per-element has_written bits.
                pair, st = divmod(g, 8)
                if st == 0:
                    ops[pair] = [
                        opp.tile([128, 512], fp32, tag=f"op{j}", name=f"op{j}_{pair}")
                        for j in range(3)
                    ]
                for th in range(2):
                    for k in range(4):
                        tt = th * 4 + k
                        j, loc = tt // 3, tt % 3
                        for b01 in range(2):
                            b = pair * 2 + b01
                            nc.tensor.matmul(
                                ops[pair][j][
                                    :,
                                    loc * 130 + b01 * 65 : loc * 130 + (b01 + 1) * 65,
                                ],
                                pts[th][:, b01, k * 128 : (k + 1) * 128],
                                kn[st][:, b, :],
                                start=(st == 0 and loc == 0 and b01 == 0),
                                stop=(st == 7),
                                skip_group_check=True,
                            )

            def emit_normalize(pair):
                # one reciprocal + one broadcast multiply per PSUM
                # bank-group (vs per-tt), writing the output tile, then
                # stream this pair's output so only pair 3's store is on
                # the kernel tail.  Pair 3's store is further split per
                # bank-group across both HWDGE queues so it streams out
                # while the remaining groups normalize.
                for j in range(3):
                    nloc = 3 if j < 2 else 2
                    opv = ops[pair][j][:, 0 : nloc * 130].rearrange(
                        "p (l b x) -> p l b x", l=nloc, b=2
                    )
                    rc = rcp.tile(
                        [128, nloc, 2, 1], fp32, tag=f"rc{j}",
                        name=f"rc_{pair}_{j}",
                    )
                    nc.vector.reciprocal(rc[:, :, :, 0], opv[:, :, :, HD])
                    nc.vector.tensor_tensor(
                        out=outt[:, j * 3 : j * 3 + nloc, pair * 2 : (pair + 1) * 2, :],
                        in0=opv[:, :, :, 0:HD],
                        in1=rc[:].to_broadcast([128, nloc, 2, HD]),
                        op=mybir.AluOpType.mult,
                    )
                    if pair == 3:
                        for b01 in range(2):
                            b = pair * 2 + b01
                            eng = nc.sync if b01 == 0 else nc.scalar
                            eng.dma_start(
                                out=out4[:, j * 3 : j * 3 + nloc, b, :],
                                in_=outt[:, j * 3 : j * 3 + nloc, b, :],
                            )
                if pair < 3:
                    for b01 in range(2):
                        b = pair * 2 + b01
                        nc.sync.dma_start(
                            out=out4[:, :, b, :],
                            in_=outt[:, :, b, :],
                        )

            prev_pts = None
            for g in range(33):
                pts = emit_front(g) if g < 32 else None
                if g >= 1:
                    emit_mm2(g - 1, prev_pts)
                    if g % 8 == 0:
                        emit_normalize(g // 8 - 1)
                prev_pts = pts

    _split_excess_waits(nc)
    return nc


_NC_CACHE = None


def _get_nc():
    global _NC_CACHE
    if _NC_CACHE is None:
        _NC_CACHE = build_nc()
    return _NC_CACHE


def kernel(queries: np.ndarray, keys: np.ndarray, attn_mask: np.ndarray) -> np.ndarray:
    assert queries.shape == (T, B, NH * HD)
    assert keys.shape == (S, B, NH * HD)
    assert attn_mask.shape == (B, T, S)

    q_bf = np.asarray(queries, np.float32).astype(BF16)  # [T, B, A]
    k_bf = np.asarray(keys, np.float32).astype(BF16)
    m_bf = np.asarray(attn_mask).astype(BF16)  # bool -> 0.0/1.0

    in_maps = []
    for c in range(N_CORES):
        qs = q_bf[:, :, c * HD : (c + 1) * HD].reshape(T, B * HD)  # [T,(b,h)]
        ks = k_bf[:, :, c * HD : (c + 1) * HD].reshape(S, B * HD)
        in_maps.append(
            {
                "qt": np.ascontiguousarray(qs.T),
                "kt": np.ascontiguousarray(ks.T),
                "knat": np.ascontiguousarray(ks),
                "maskt": np.ascontiguousarray(m_bf[c].T),
            }
        )

    nc = _get_nc()
    res = run_bass_kernel_spmd(nc, in_maps, core_ids=list(range(N_CORES)))
    kernel.last_results = res

    outp = np.empty((T, B, NH * HD), np.float32)
    for c in range(N_CORES):
        # device output is [B, T, HD]; put back as [T, B, HD-slice]
        outp[:, :, c * HD : (c + 1) * HD] = res.results[c]["out"].transpose(1, 0, 2)
    return outp
